# revision 1
# baseline (speedup 1.0000x reference)
"""GraphConvolution (GCN layer) Trainium2 kernel, 8-core SPMD.

  support = relu(feature @ W)                       [N, D]
  msgs    = edge_vals[:, None] * support[col]       [E, D]
  agg     = segment_sum(msgs, row, N)
  out     = agg + support + bias

Sharding: nodes (feature rows / output rows / segment destinations) are
split across 8 cores; edges are partitioned by destination row.  W and
bias are replicated.  Each core computes support for its node shard, the
shards are AllGathered so every core can gather arbitrary source rows,
then each core computes segment sums for its destination shard.

Segment-sum on device: edges are grouped (on host) into 128-edge chunks
whose destinations fall inside a 128-row window.  For each chunk the
device builds sel[e, d] = (dloc[e] == d) * val[e] with a single fused
DVE tensor_scalar op (iota is_equal dloc, then mult val), and one PE
matmul accumulates sel.T @ gathered_rows into the window's PSUM tile.
"""

import numpy as np

P = 128


def _pack_edges(rows, cols, vals, n_cores, nodes_pc_raw, nodes_pc, nb):
    """Group edges by (dest core, 128-dest window), pack into 128-edge chunks.

    Returns per-core [P, n_chunks] arrays: idx (padded global source id),
    val (edge weight, 0 for padding), dloc (dest offset inside window).
    The chunk schedule (chunks per window) is shared by all cores.
    """
    rows = rows.astype(np.int64)
    cols = cols.astype(np.int64)
    core = rows // nodes_pc_raw
    loc = rows - core * nodes_pc_raw
    win = loc // P
    wloc = loc - win * P
    cols_pad = (cols // nodes_pc_raw) * nodes_pc + (cols % nodes_pc_raw)

    counts = np.zeros((n_cores, nb), np.int64)
    np.add.at(counts, (core, win), 1)
    kw = np.maximum(1, -(-counts.max(axis=0) // P))  # chunks per window
    cs = np.zeros(nb + 1, np.int64)
    cs[1:] = np.cumsum(kw)
    n_chunks = int(cs[-1])

    idx = np.zeros((n_cores, P, n_chunks), np.int32)
    val = np.zeros((n_cores, P, n_chunks), np.float32)
    dloc = np.zeros((n_cores, P, n_chunks), np.float32)

    gkey = core * nb + win
    order = np.argsort(gkey, kind="stable")
    sg = gkey[order]
    if len(sg):
        starts = np.ones(len(sg), bool)
        starts[1:] = sg[1:] != sg[:-1]
        start_idx = np.flatnonzero(starts)
        seg_len = np.diff(np.append(start_idx, len(sg)))
        rank = np.arange(len(sg)) - np.repeat(start_idx, seg_len)
        oc = core[order]
        ow = win[order]
        col_idx = cs[ow] + rank // P
        row_idx = rank % P
        idx[oc, row_idx, col_idx] = cols_pad[order]
        val[oc, row_idx, col_idx] = vals[order]
        dloc[oc, row_idx, col_idx] = wloc[order]
    return idx, val, dloc, [int(k) for k in kw], cs, n_chunks


def _build_program(n_cores, nodes_pc, f_dim, d_dim, kw, cs, n_chunks):
    import concourse.bass as bass
    import concourse.mybir as mybir
    import concourse.tile as tile
    from concourse import bacc

    dt = mybir.dt
    kf = f_dim // P
    nb = nodes_pc // P
    n_pad = nodes_pc * n_cores

    nc = bacc.Bacc(None, num_devices=n_cores)
    feat_t = nc.declare_dram_parameter("featT", [f_dim, nodes_pc], dt.float32, isOutput=False)
    w_in = nc.declare_dram_parameter("Wm", [f_dim, d_dim], dt.float32, isOutput=False)
    bias_in = nc.declare_dram_parameter("bias_rep", [P, d_dim], dt.float32, isOutput=False)
    idx_in = nc.declare_dram_parameter("idx", [P, n_chunks], dt.int32, isOutput=False)
    val_in = nc.declare_dram_parameter("val", [P, n_chunks], dt.float32, isOutput=False)
    dloc_in = nc.declare_dram_parameter("dloc", [P, n_chunks], dt.float32, isOutput=False)
    out_t = nc.declare_dram_parameter("out", [nodes_pc, d_dim], dt.float32, isOutput=True)

    support_c = nc.dram_tensor("support_c", [nodes_pc, d_dim], dt.float32)
    support_full = nc.dram_tensor(
        "support_full", [n_pad, d_dim], dt.float32, addr_space="Shared"
    )

    kmax = max(kw)

    with tile.TileContext(nc) as tc:
        with (
            tc.tile_pool(name="const", bufs=1) as cpool,
            tc.tile_pool(name="sup", bufs=1) as spool,
            tc.tile_pool(name="meta", bufs=1) as mpool,
            tc.tile_pool(name="work", bufs=4) as wpool,
            tc.tile_pool(name="gath", bufs=3) as gpool,
            tc.tile_pool(name="outp", bufs=4) as opool,
            tc.tile_pool(name="psA", bufs=2, space="PSUM") as ppool_a,
            tc.tile_pool(name="psB", bufs=2, space="PSUM") as ppool_b,
        ):
            w_sb = cpool.tile([P, kf * d_dim], dt.float32)
            for k in range(kf):
                nc.sync.dma_start(
                    out=w_sb[:, k * d_dim : (k + 1) * d_dim],
                    in_=w_in[k * P : (k + 1) * P, :],
                )
            bias_sb = cpool.tile([P, d_dim], dt.float32)
            nc.sync.dma_start(out=bias_sb[:], in_=bias_in[:])
            iota_f = cpool.tile([P, P], dt.float32)
            nc.gpsimd.iota(
                iota_f[:], pattern=[[1, P]], base=0, channel_multiplier=0,
                allow_small_or_imprecise_dtypes=True,
            )

            idx_sb = mpool.tile([P, n_chunks], dt.int32)
            nc.sync.dma_start(out=idx_sb[:], in_=idx_in[:])
            val_sb = mpool.tile([P, n_chunks], dt.float32)
            nc.sync.dma_start(out=val_sb[:], in_=val_in[:])
            dloc_sb = mpool.tile([P, n_chunks], dt.float32)
            nc.sync.dma_start(out=dloc_sb[:], in_=dloc_in[:])

            # Consolidate engine vector clocks against the persistent tiles
            # so hot-loop instructions need at most one semaphore wait
            # (TensorScalarPtr tolerates very few sync waits on TRN2).
            warm = cpool.tile([P, 4], dt.float32)
            nc.vector.tensor_copy(out=warm[:, 0:1], in_=iota_f[:, :1])
            nc.vector.tensor_copy(out=warm[:, 1:2], in_=dloc_sb[:, :1])
            nc.vector.tensor_copy(out=warm[:, 2:3], in_=val_sb[:, :1])
            nc.vector.tensor_copy(out=warm[:, 3:4], in_=bias_sb[:, :1])
            warm2 = cpool.tile([P, 2], dt.float32)
            nc.scalar.copy(out=warm2[:, 0:1], in_=val_sb[:, :1])
            nc.scalar.copy(out=warm2[:, 1:2], in_=iota_f[:, :1])

            sup_sb = spool.tile([P, nb * d_dim], dt.float32)

            # Phase A: support = relu(feature @ W) for the local node shard.
            for b in range(nb):
                ps = ppool_a.tile([P, d_dim], dt.float32, tag="mm")
                for k in range(kf):
                    ft = wpool.tile([P, P], dt.float32, tag="ft")
                    nc.sync.dma_start(
                        out=ft[:],
                        in_=feat_t[k * P : (k + 1) * P, b * P : (b + 1) * P],
                    )
                    nc.tensor.matmul(
                        ps[:],
                        lhsT=ft[:],
                        rhs=w_sb[:, k * d_dim : (k + 1) * d_dim],
                        start=(k == 0),
                        stop=(k == kf - 1),
                    )
                nc.scalar.activation(
                    out=sup_sb[:, b * d_dim : (b + 1) * d_dim],
                    in_=ps[:],
                    func=mybir.ActivationFunctionType.Relu,
                )
                nc.sync.dma_start(
                    out=support_c[b * P : (b + 1) * P, :],
                    in_=sup_sb[:, b * d_dim : (b + 1) * d_dim],
                )

            nc.gpsimd.collective_compute(
                "AllGather",
                mybir.AluOpType.bypass,
                replica_groups=[list(range(n_cores))],
                ins=[support_c[:]],
                outs=[support_full[:]],
            )

            # Phase B: per 128-dest window, gather + scaled-one-hot matmul.
            for w in range(nb):
                k_here = kw[w]
                c0 = int(cs[w])
                ps = ppool_b.tile([P, d_dim], dt.float32, tag="agg")
                for j in range(k_here):
                    msgs = gpool.tile([P, d_dim], dt.float32, tag="msgs")
                    nc.gpsimd.indirect_dma_start(
                        out=msgs[:],
                        out_offset=None,
                        in_=support_full[:, :],
                        in_offset=bass.IndirectOffsetOnAxis(
                            ap=idx_sb[:, c0 + j : c0 + j + 1], axis=0
                        ),
                    )
                    selv = wpool.tile([P, P], dt.float32, tag="selv")
                    nc.vector.tensor_scalar(
                        out=selv[:],
                        in0=iota_f[:],
                        scalar1=dloc_sb[:, c0 + j : c0 + j + 1],
                        scalar2=None,
                        op0=mybir.AluOpType.is_equal,
                    )
                    ms = wpool.tile([P, d_dim], dt.float32, tag="ms")
                    nc.scalar.mul(
                        out=ms[:],
                        in_=msgs[:],
                        mul=val_sb[:, c0 + j : c0 + j + 1],
                    )
                    nc.tensor.matmul(
                        ps[:],
                        lhsT=selv[:],
                        rhs=ms[:],
                        start=(j == 0),
                        stop=(j == k_here - 1),
                    )
                ob = opool.tile([P, d_dim], dt.float32, tag="ob")
                nc.vector.tensor_add(
                    out=ob[:], in0=ps[:], in1=sup_sb[:, w * d_dim : (w + 1) * d_dim]
                )
                nc.vector.tensor_add(out=ob[:], in0=ob[:], in1=bias_sb[:])
                nc.sync.dma_start(out=out_t[w * P : (w + 1) * P, :], in_=ob[:])

    nc.compile()
    return nc


def kernel_impl(feature, W, bias, edge_vals, edge_index, n_cores=8, trace=False):
    from concourse.bass_utils import run_bass_kernel_spmd

    n, f_dim = feature.shape
    d_dim = W.shape[1]
    assert n % n_cores == 0
    nodes_pc_raw = n // n_cores
    nb = -(-nodes_pc_raw // P)
    nodes_pc = nb * P

    idx, val, dloc, kw, cs, n_chunks = _pack_edges(
        edge_index[0], edge_index[1], edge_vals.astype(np.float32),
        n_cores, nodes_pc_raw, nodes_pc, nb,
    )

    nc = _build_program(n_cores, nodes_pc, f_dim, d_dim, kw, cs, n_chunks)

    bias_rep = np.ascontiguousarray(
        np.broadcast_to(bias.astype(np.float32), (P, d_dim))
    )
    w_c = np.ascontiguousarray(W.astype(np.float32))
    in_maps = []
    for c in range(n_cores):
        shard = feature[c * nodes_pc_raw : (c + 1) * nodes_pc_raw].astype(np.float32)
        if nodes_pc != nodes_pc_raw:
            shard = np.pad(shard, ((0, nodes_pc - nodes_pc_raw), (0, 0)))
        feat_t = np.ascontiguousarray(shard.T)
        in_maps.append(
            {
                "featT": feat_t,
                "Wm": w_c,
                "bias_rep": bias_rep,
                "idx": np.ascontiguousarray(idx[c]),
                "val": np.ascontiguousarray(val[c]),
                "dloc": np.ascontiguousarray(dloc[c]),
            }
        )

    res = run_bass_kernel_spmd(nc, in_maps, list(range(n_cores)), trace=trace)
    out = np.concatenate(
        [res.results[c]["out"][:nodes_pc_raw] for c in range(n_cores)], axis=0
    )
    if trace:
        return out, res
    return out


def kernel(feature, W, bias, edge_vals, edge_index):
    return kernel_impl(
        np.asarray(feature), np.asarray(W), np.asarray(bias),
        np.asarray(edge_vals), np.asarray(edge_index),
    )

